# revision 1
# baseline (speedup 1.0000x reference)
"""Multi-head attention block (B=4, L=S=2048, D=P=1024, H=8) on 8 TRN2 cores.

Sharding: core c = 2*b + g handles batch b and head-group g (4 heads).
  - Wq/Wk/Wv column slice [1024, 512], Wo row slice [512, 1024].
  - Each core computes a partial output [2048, 1024] (its head-group's
    contribution through Wo); the host sums the two partials per batch and
    adds bo (the tensor-parallel all-reduce done at unshard time).

The host casts activations/weights to bf16 (and pre-transposes the small
weight slices) when building the per-core input maps; all device matmuls run
bf16 with fp32 PSUM accumulation.

Per-core kernel:
  1. X (xq/xk/xv, bf16 in DRAM) loaded via batched xbar transposing DMAs
     straight into feature-major X^T chunk tensors (one instruction per
     128-token tile fills all 8 k-stripes). Weights load the same way from
     transposed host arrays, so the whole prep window stays in
     xbar-transpose mode (mode switches serialize the DMA domain).
  2. q^T, k^T [512, 2048] feature-major (lhsT=W, rhs=X^T) + bias;
     v [2048, 512] token-major (lhsT=X^T, rhs=W) + bias, stored interleaved
     with a ones-column per head (v_aug [2048, 4*132]) so the attention
     row-sums fall out of the ctx matmul for free.
  3. Per (l-half, head): scores^T tiles [s=128, l=1024] on PE; exp on ACT
     (scale=1/sqrt(128)) -> E^T bf16; ctx[l, 129] accumulated over s in
     PSUM accumulators packed 3-per-bank (with a bank-wide zeroing matmul
     first, since start=True clears has_written for the whole bank); col
     128 is the softmax denominator; normalize with DVE reciprocal+scale.
  4. ctx_norm [2048, 512] bf16 -> xbar-transpose -> ctx^T; out-projection
     (lhsT=ctx^T, rhs=Wo) -> partial out f32 [2048, 1024] -> DRAM.

Measured: ~333-337 us HW exec (neuron-profile), rel err ~5.1e-3 vs the f32
reference (first correct serial version was 1078 us).
"""

import sys

sys.path.insert(0, "/opt/trn_rl_repo")

import math

import numpy as np

import concourse.bass as bass
import concourse.tile as tile
from concourse import bacc, mybir
from concourse.bass_utils import run_bass_kernel_spmd

F32 = mybir.dt.float32
BF16 = mybir.dt.bfloat16

TOK = 2048          # tokens per core (one batch), 16 tiles of 128
DF = 1024           # model dim, 8 k-tiles of 128
PF = 512            # per-core projection width (4 heads x 128)
NHEAD = 4           # heads per core
EH = 128            # head dim
VSTRIDE = 132       # v_aug per-head stride (128 v cols + 1 ones col + 3 pad)
SCALE = 1.0 / math.sqrt(128.0)

T16 = TOK // 128    # 16 token tiles
K8 = DF // 128      # 8 feature k-tiles
M4 = PF // 128      # 4 outf tiles == heads
N4 = TOK // 512     # 4 token chunks of 512
LHALF = 2           # two l-halves of 1024


def _build(debug_taps=False):
    nc = bacc.Bacc("TRN2", target_bir_lowering=False, debug=False, num_devices=8)

    xq = nc.dram_tensor("xq", [TOK, DF], BF16, kind="ExternalInput")
    xk = nc.dram_tensor("xk", [TOK, DF], BF16, kind="ExternalInput")
    xv = nc.dram_tensor("xv", [TOK, DF], BF16, kind="ExternalInput")
    wq = nc.dram_tensor("wq", [PF, DF], BF16, kind="ExternalInput")  # Wq slice, transposed
    wk = nc.dram_tensor("wk", [PF, DF], BF16, kind="ExternalInput")  # Wk slice, transposed
    wv = nc.dram_tensor("wv", [PF, DF], BF16, kind="ExternalInput")  # Wv slice, transposed
    wo = nc.dram_tensor("wo", [DF, PF], BF16, kind="ExternalInput")  # Wo slice, transposed
    bq = nc.dram_tensor("bq", [PF], F32, kind="ExternalInput")
    bk = nc.dram_tensor("bk", [PF], F32, kind="ExternalInput")
    bv = nc.dram_tensor("bv", [PF], F32, kind="ExternalInput")
    out = nc.dram_tensor("out", [TOK, DF], F32, kind="ExternalOutput")

    with tile.TileContext(nc) as tc:
        with tc.tile_pool(name="sb", bufs=1) as sb, \
             tc.tile_pool(name="ps", bufs=1, space="PSUM") as ps:

            # ---- biases -------------------------------------------------
            # bq/bk as [128, 4] f32: column m = bias slice for outf tile m.
            bq_sb = sb.tile([128, M4], F32, tag="bq_sb")
            bk_sb = sb.tile([128, M4], F32, tag="bk_sb")
            nc.gpsimd.dma_start(bq_sb[:], bq[:].rearrange("(m p) -> p m", p=128))
            nc.gpsimd.dma_start(bk_sb[:], bk[:].rearrange("(m p) -> p m", p=128))
            # bv broadcast to all 128 partitions via K=1 fp32 matmul.
            bv_row = sb.tile([1, PF], F32, tag="bv_row")
            nc.gpsimd.dma_start(bv_row[:], bv[:].rearrange("(o n) -> o n", o=1))
            ones1 = sb.tile([1, 128], F32, tag="ones1")
            nc.vector.memset(ones1[:], 1.0)
            bvb_ps = ps.tile([128, 512], F32, tag="out_ps", bufs=1)
            nc.tensor.matmul(bvb_ps[:], ones1[:], bv_row[:], start=True, stop=True)
            bvb = sb.tile([128, PF], F32, tag="bvb")
            nc.vector.tensor_copy(bvb[:], bvb_ps[:])

            # ---- weights -> bf16 (SWDGE casting DMA straight from HBM) --
            # weight layout on SBUF: w*_sb[p, 512k + outf] = W[128k + p, outf]
            # (feature-major); loaded via transposing DMAs from the host-side
            # TRANSPOSED weight slices so the whole prep window stays in
            # xbar-transpose mode (mode switches serialize the DMA domain).
            wv_sb = sb.tile([128, K8 * PF], BF16, tag="wv_sb", name="wv_sb")
            wq_sb = sb.tile([128, K8 * PF], BF16, tag="wq_sb", name="wq_sb")
            wk_sb = sb.tile([128, K8 * PF], BF16, tag="wk_sb", name="wk_sb")
            wo_sb = sb.tile([128, M4 * DF], BF16, tag="wo_sb", name="wo_sb")

            def load_weight_T(dst, dram, width):
                # dram: [width(out-dim), DF or PF (in-dim)] transposed slice
                nk = dram.shape[1] // 128
                d3 = dst.rearrange("p (k o) -> p k o", k=nk)
                for j in range(width // 128):
                    nc.sync.dma_start(
                        d3[:, :, 128 * j:128 * (j + 1)],
                        dram[128 * j:128 * (j + 1), :],
                        transpose=True,
                    )


            # ---- persistent activation tensors -------------------------
            qT = [sb.tile([128, TOK], BF16, tag=f"qT{m}", name=f"qT{m}") for m in range(M4)]
            kT = [sb.tile([128, TOK], BF16, tag=f"kT{m}", name=f"kT{m}") for m in range(M4)]
            v_aug = [sb.tile([128, NHEAD * VSTRIDE], BF16, tag=f"va{t}", name=f"va{t}")
                     for t in range(T16)]
            for t in range(T16):
                nc.vector.memset(v_aug[t][:], 1.0)
            # ctxT: 4 chunk tensors of 512 tokens, feature f-tile at
            # cols [512f, 512f+512) within each chunk
            ctxTc = [sb.tile([128, M4 * 512], BF16, tag=f"cT{c}", name=f"cT{c}")
                     for c in range(N4)]
            # zero rhs used to reset ctx-accumulator PSUM banks (start=True
            # clears has_written for the WHOLE bank, so packed accumulators
            # must share a single bank-wide clear)
            zeros_bf = sb.tile([128, 512], BF16, tag="zeros_bf")
            nc.vector.memset(zeros_bf[:], 0.0)

            # ---- X -> bf16 -> X^T, then projection ----------------------
            # xT is one [128, 8*2048] tensor: k-tile k at cols [2048k, 2048k+2048),
            # i.e. xT[p, 2048k + tok] = X[tok, 128k + p]. One batched xbar
            # transpose instruction per token tile fills all 8 k-tiles.
            def prep_xT(*x_drams):
                # transposing loads straight from DRAM (bf16): one xbar
                # instruction per token tile fills all 8 k-tile stripes.
                # xT is split into 4 chunk tensors of 512 tokens each
                # ([128, 8k x 512]) so transpose writers of chunk c only
                # serialize against readers of chunk c. Multiple inputs are
                # interleaved chunk by chunk.
                all_xTc = [[sb.tile([128, K8 * 512], BF16, tag="xT", bufs=8,
                                    name=f"xTc{i}_{c}") for c in range(N4)]
                           for i in range(len(x_drams))]
                for c in range(N4):
                    for i, x_dram in enumerate(x_drams):
                        xT3 = all_xTc[i][c].rearrange(
                            "p (k tok) -> p k tok", tok=512)
                        for tt in range(4):
                            t = 4 * c + tt
                            nc.sync.dma_start(
                                xT3[:, :, 128 * tt:128 * (tt + 1)],
                                x_dram[128 * t:128 * (t + 1), :],
                                transpose=True,
                            )

                def make_ap(xTc):
                    def xt_ap(k, lo, width):
                        cc, off = divmod(lo, 512)
                        assert off + width <= 512
                        return xTc[cc][:, 512 * k + off:512 * k + off + width]
                    return xt_ap
                aps = [make_ap(xTc) for xTc in all_xTc]
                return aps if len(aps) > 1 else aps[0]

            def proj_T(xT, w_sb, b_sb, dstT, m):
                # dstT[m][:, n] = (X @ W + b)^T, feature-major
                for n in range(N4):
                    pst = ps.tile([128, 512], F32, tag="att_ps", bufs=2)
                    for k in range(K8):
                        nc.tensor.matmul(
                            pst[:],
                            w_sb[:, PF * k + 128 * m:PF * k + 128 * (m + 1)],
                            xT(k, 512 * n, 512),
                            start=(k == 0), stop=(k == K8 - 1),
                        )
                    nc.vector.tensor_scalar_add(
                        dstT[m][:, 512 * n:512 * (n + 1)], pst[:],
                        b_sb[:, m:m + 1],
                    )

            load_weight_T(wv_sb, wv, PF)
            xvT = prep_xT(xv)
            # v token-major, interleaved with ones columns
            for t in range(T16):
                pst = ps.tile([128, 512], F32, tag="att_ps", bufs=2)
                for k in range(K8):
                    nc.tensor.matmul(
                        pst[:],
                        xvT(k, 128 * t, 128),
                        wv_sb[:, PF * k:PF * (k + 1)],
                        start=(k == 0), stop=(k == K8 - 1),
                    )
                for h in range(NHEAD):
                    nc.vector.tensor_add(
                        v_aug[t][:, VSTRIDE * h:VSTRIDE * h + 128],
                        pst[:, 128 * h:128 * (h + 1)],
                        bvb[:, 128 * h:128 * (h + 1)],
                    )
            del xvT
            # q/k prep interleaved by chunk, projections interleaved by head,
            # so head-0 attention (ACT-bound) can start during the
            # remaining projections.
            load_weight_T(wq_sb, wq, PF)
            xqT = prep_xT(xq)
            for m in range(M4):
                proj_T(xqT, wq_sb, bq_sb, qT, m)
            del xqT
            load_weight_T(wk_sb, wk, PF)
            xkT = prep_xT(xk)
            for m in range(M4):
                proj_T(xkT, wk_sb, bk_sb, kT, m)
            del xkT
            load_weight_T(wo_sb, wo, DF)

            if debug_taps:
                qT_out = nc.dram_tensor("qT_out", [PF, TOK], F32, kind="ExternalOutput")
                kT_out = nc.dram_tensor("kT_out", [PF, TOK], F32, kind="ExternalOutput")
                va_out = nc.dram_tensor("va_out", [TOK, NHEAD * VSTRIDE], F32,
                                        kind="ExternalOutput")
                ctxn_out = nc.dram_tensor("ctxn_out", [TOK, PF], F32,
                                          kind="ExternalOutput")
                eT_out = nc.dram_tensor("eT_out", [TOK, 1024], F32,
                                        kind="ExternalOutput")
                acc_out = nc.dram_tensor("acc_out", [1024, 129], F32,
                                         kind="ExternalOutput")
                for m in range(M4):
                    for n in range(N4):
                        dbg = sb.tile([128, 512], F32, tag="dbg", bufs=2, name="dbg")
                        nc.vector.tensor_copy(dbg[:], qT[m][:, 512*n:512*(n+1)])
                        nc.gpsimd.dma_start(
                            qT_out[128*m:128*(m+1), 512*n:512*(n+1)], dbg[:])
                        dbg2 = sb.tile([128, 512], F32, tag="dbg", bufs=2, name="dbg2")
                        nc.vector.tensor_copy(dbg2[:], kT[m][:, 512*n:512*(n+1)])
                        nc.gpsimd.dma_start(
                            kT_out[128*m:128*(m+1), 512*n:512*(n+1)], dbg2[:])
                for t in range(T16):
                    dbg3 = sb.tile([128, NHEAD * VSTRIDE], F32, tag="dbg3",
                                   bufs=2, name="dbg3")
                    nc.vector.tensor_copy(dbg3[:], v_aug[t][:])
                    nc.gpsimd.dma_start(va_out[128*t:128*(t+1), :], dbg3[:])

            # ---- attention + out-projection ----------------------------
            # Emission order: att(lh0,h0..h3), transposes(lh0), att(lh1,h0),
            # outproj(lh0), att(lh1,h1..h3), transposes(lh1), outproj(lh1).
            # Placing outproj(lh0) behind att(lh1,h0) in program order keeps
            # its PE matmuls as low-priority filler during the ACT-bound
            # attention stretch instead of starving the exp pipeline at the
            # l-half boundary.
            ctxn = [None] * T16

            def attention_head(lh, h):
                # 8 ctx accumulators [128, 129] packed 3 per PSUM bank
                cps = [ps.tile([128, 512], F32, tag="ctx_ps", bufs=3,
                               name=f"cps{lh}_{h}_{_}") for _ in range(3)]
                # bank-wide clear: one zeroing matmul per accumulator bank
                for i in range(3):
                    nc.tensor.matmul(cps[i][:], zeros_bf[0:128, 0:128],
                                     zeros_bf[:], start=True, stop=False,
                                     skip_group_check=True)

                def acc(j):
                    i, jj = divmod(j, 3)
                    return cps[i][:, 129 * jj:129 * jj + 129]

                for s in range(T16):
                    sc = ps.tile([128, 1024], F32, tag="att_ps", bufs=2)
                    for c2 in range(2):
                        nc.tensor.matmul(
                            sc[:, 512 * c2:512 * (c2 + 1)],
                            kT[h][:, 128 * s:128 * (s + 1)],
                            qT[h][:, 1024 * lh + 512 * c2:
                                     1024 * lh + 512 * (c2 + 1)],
                            start=True, stop=True,
                        )
                    eT = sb.tile([128, 1024], BF16, tag="eT", bufs=4)
                    nc.scalar.activation(
                        eT[:], sc[:], mybir.ActivationFunctionType.Exp,
                        scale=SCALE,
                    )
                    for j in range(8):
                        nc.tensor.matmul(
                            acc(j),
                            eT[:, 128 * j:128 * (j + 1)],
                            v_aug[s][:, VSTRIDE * h:VSTRIDE * h + 129],
                            start=False, stop=(s == T16 - 1),
                            skip_group_check=True,
                        )
                for j in range(8):
                    t = 8 * lh + j
                    if ctxn[t] is None:
                        ctxn[t] = sb.tile([128, PF], BF16, tag="ctxn",
                                          bufs=17, name=f"ctxn{t}")
                    rs = sb.tile([128, 1], F32, tag="rs", bufs=4)
                    nc.vector.reciprocal(rs[:], acc(j)[:, 128:129])
                    nc.vector.tensor_scalar_mul(
                        ctxn[t][:, 128 * h:128 * (h + 1)],
                        acc(j)[:, 0:128], rs[:, 0:1],
                    )

            def ctx_transposes(lh):
                for j in range(8):
                    t = 8 * lh + j
                    cc, ttt = divmod(t, 4)
                    cT3 = ctxTc[cc].rearrange("p (f tok) -> p f tok", tok=512)
                    nc.sync.dma_start(
                        cT3[:, :, 128 * ttt:128 * (ttt + 1)], ctxn[t][:],
                        transpose=True,
                    )
                    ctxn[t] = None

            def outproj(lh, ps_tag="out_ps", ps_bufs=1):
                for j in range(8):
                    t = 8 * lh + j
                    for n2 in range(2):
                        pso = ps.tile([128, 512], F32, tag=ps_tag, bufs=ps_bufs)
                        for kf in range(M4):
                            nc.tensor.matmul(
                                pso[:],
                                ctxTc[t // 4][:, 512 * kf + 128 * (t % 4):
                                              512 * kf + 128 * (t % 4) + 128],
                                wo_sb[:, DF * kf + 512 * n2:DF * kf + 512 * (n2 + 1)],
                                start=(kf == 0), stop=(kf == M4 - 1),
                            )
                        osb = sb.tile([128, 512], F32, tag="osb", bufs=4)
                        if lh == 1 and (2 * j + n2) % 2 == 0:
                            nc.scalar.copy(osb[:], pso[:])
                        else:
                            nc.vector.tensor_copy(osb[:], pso[:])
                        nc.gpsimd.dma_start(
                            out[128 * t:128 * (t + 1), 512 * n2:512 * (n2 + 1)],
                            osb[:],
                        )

            for h in range(NHEAD):
                attention_head(0, h)
            ctx_transposes(0)
            attention_head(1, 0)
            outproj(0)
            for h in range(1, NHEAD):
                attention_head(1, h)
            ctx_transposes(1)
            outproj(1, ps_tag="ctx_ps", ps_bufs=3)

    nc.finalize()
    return nc


_NC_CACHE = None


def _get_nc():
    global _NC_CACHE
    if _NC_CACHE is None:
        _NC_CACHE = _build()
    return _NC_CACHE


def _make_in_maps(queries, keys, values, Wq, bq, Wk, bk, Wv, bv, Wo):
    import ml_dtypes

    def c(a):
        return np.ascontiguousarray(a)

    def cb(a):
        return np.ascontiguousarray(np.asarray(a, np.float32).astype(ml_dtypes.bfloat16))
    in_maps = []
    for core in range(8):
        b, g = divmod(core, 2)
        sl = slice(512 * g, 512 * (g + 1))
        in_maps.append({
            "xq": cb(queries[b]),
            "xk": cb(keys[b]),
            "xv": cb(values[b]),
            "wq": cb(Wq[:, sl].T), "wk": cb(Wk[:, sl].T), "wv": cb(Wv[:, sl].T),
            "wo": cb(Wo[sl, :].T),
            "bq": c(bq[sl]), "bk": c(bk[sl]), "bv": c(bv[sl]),
        })
    return in_maps


def _run(trace=False, **inputs):
    arrs = {k: np.asarray(v, dtype=np.float32) for k, v in inputs.items()}
    nc = _get_nc()
    in_maps = _make_in_maps(
        arrs["queries"], arrs["keys"], arrs["values"],
        arrs["Wq"], arrs["bq"], arrs["Wk"], arrs["bk"],
        arrs["Wv"], arrs["bv"], arrs["Wo"],
    )
    res = run_bass_kernel_spmd(nc, in_maps, core_ids=list(range(8)), trace=trace)
    bo = arrs["bo"]
    full = np.empty((4, TOK, DF), np.float32)
    for b in range(4):
        full[b] = res.results[2 * b]["out"] + res.results[2 * b + 1]["out"] + bo
    return full, res


def kernel(**inputs) -> np.ndarray:
    full, _ = _run(trace=False, **inputs)
    return full

